# revision 30
# baseline (speedup 1.0000x reference)
"""Causal MHSA (B=2, S=2048, D=1024, H=16, RoPE) on 8 TRN2 NeuronCores.

Sharding: each core owns 2 heads x both batches (tensor parallel on
heads). Attention runs in transposed ("scores^T") layout so no on-device
transposes of Q/K/attn are needed; softmax denominators ride as an extra
ones-column in V. An 8-rank AllToAll re-shards from head-parallel to
sequence-parallel, after which each core runs the output projection for
its (batch, seq-slice) and emits a [512, 1024] output slice.

All matmul operands are bf16 (fp32 accumulation in PSUM); q/k stay bf16
through RoPE so the per-head score matmuls run row-grouped and
concurrently at full bf16 rate. V is projected directly into [seq, dim]
layout (x s-chunks as the stationary operand), removing PE transposes.
Projection chunks and attention q-chunks are interleaved in program
order so the PE has dense work while the scalar engine (exp) grinds
through softmax; attention outputs are staged into the AllToAll input
eagerly per (batch, q-chunk) so the collective triggers immediately
after the last chunk. Input DMAs are split across the Sync/Scalar/GpSimd
queues and ordered so the first projection chunk starts early.
"""

import os
import numpy as np

import concourse.bass as bass
import concourse.bacc as bacc
import concourse.mybir as mybir
import concourse.tile as tile
from concourse.bass_utils import run_bass_kernel_spmd

B, S, D, H, DK = 2, 2048, 1024, 16, 64
NCORES = 8
HL = 2            # heads per core
DLOC = HL * DK    # 128 local head dims
SC = 512          # q-chunk / moving free dim
NQC = S // SC     # 4 q-chunks
NKT = S // 128    # 16 k-tiles
NET = D // 128    # 8 e-tiles (contraction over embedding)
SSL = S // 4      # 512-row seq slice per rank (within a batch)
VROW = 65         # 64 dims + denominator ones-row
ROPE_THETA = 10000.0

f32 = mybir.dt.float32
bf16 = mybir.dt.bfloat16

LAST_EXEC_NS = {"ns": None}
_CACHE = {}


def _build_nc():
    nc = bacc.Bacc(
        "TRN2", target_bir_lowering=False, debug=False, num_devices=NCORES
    )

    xT = [
        nc.dram_tensor(f"xT{b}", [D, S], bf16, kind="ExternalInput").ap()
        for b in range(B)
    ]
    wq = nc.dram_tensor("wq", [D, DLOC], bf16, kind="ExternalInput").ap()
    wk = nc.dram_tensor("wk", [D, DLOC], bf16, kind="ExternalInput").ap()
    wv = nc.dram_tensor("wv", [D, DLOC], bf16, kind="ExternalInput").ap()
    woT = nc.dram_tensor("woT", [D, D], bf16, kind="ExternalInput").ap()
    cos_d = nc.dram_tensor("cos_t", [128, S], bf16, kind="ExternalInput").ap()
    sin_d = nc.dram_tensor("sin_t", [128, S], bf16, kind="ExternalInput").ap()
    perm_d = nc.dram_tensor("perm128", [128, 128], bf16,
                            kind="ExternalInput").ap()
    mask_d = nc.dram_tensor("mask256", [128, 256], bf16,
                            kind="ExternalInput").ap()
    sel_d = nc.dram_tensor("sel16", [16, 1024], bf16, kind="ExternalInput").ap()
    out = nc.dram_tensor("out", [SSL, D], f32, kind="ExternalOutput").ap()

    with tile.TileContext(nc) as tc:
        _body(nc, tc, xT, wq, wk, wv, woT, cos_d, sin_d, perm_d, mask_d,
              sel_d, out)

    nc.compile()
    return nc


def _body(nc, tc, xT, wq, wk, wv, woT, cos_d, sin_d, perm_d, mask_d,
          sel_d, out):
    Exp = mybir.ActivationFunctionType.Exp

    with (
        tc.tile_pool(name="const", bufs=1) as cpool,
        tc.tile_pool(name="xw", bufs=1) as xwpool,
        tc.tile_pool(name="qkv", bufs=1) as qkvp,
        tc.tile_pool(name="dram", bufs=1, space="DRAM") as dpool,
    ):
        # ---------------- input DMAs, spread across queues ----------------
        # gpsimd queue: q/k weights first (needed by proj chunk 0), then
        # rope tables. The scalar queue is kept free of early DMA issues so
        # the exp stream starts as soon as the first scores land.
        w_sb = {}
        cos_sb = cpool.tile([128, S], bf16, tag="cos")
        sin_sb = cpool.tile([128, S], bf16, tag="sin")
        for et in range(NET):
            t = cpool.tile([128, DLOC], bf16, tag=f"wq{et}", name=f"w_q_{et}")
            nc.gpsimd.dma_start(t[:], wq[128 * et:128 * et + 128, :])
            w_sb["q", et] = t
        # rope tables arrive chunk 0 first: only 2x128KB sits on the
        # critical path to the first scores instead of the full 1MB.
        nc.gpsimd.dma_start(cos_sb[:, 0:SC], cos_d[:, 0:SC])
        nc.gpsimd.dma_start(sin_sb[:, 0:SC], sin_d[:, 0:SC])
        perm_sb = cpool.tile([128, 128], bf16, tag="perm")
        nc.gpsimd.dma_start(perm_sb[:], perm_d[:])
        for et in range(NET):
            t = cpool.tile([128, DLOC], bf16, tag=f"wk{et}", name=f"w_k_{et}")
            nc.gpsimd.dma_start(t[:], wk[128 * et:128 * et + 128, :])
            w_sb["k", et] = t
        for cc_ in range(1, NQC):
            csl = slice(SC * cc_, SC * cc_ + SC)
            nc.gpsimd.dma_start(cos_sb[:, csl], cos_d[:, csl])
            nc.gpsimd.dma_start(sin_sb[:, csl], sin_d[:, csl])

        # sync queue: batch-0 x in half-row chunks (proj chunk 0 early).
        xts = {}
        for b in range(B):
            for et in range(NET):
                t = xwpool.tile([128, S], bf16, tag=f"xt{b}{et}",
                                name=f"xt{b}_{et}")
                xts[b, et] = t
        # chunk 0 alone first: it is the only x data on the critical path to
        # the first scores/exp, and the rings are bandwidth-bound early.
        for et in range(NET):
            nc.sync.dma_start(xts[0, et][:, 0:512],
                              xT[0][128 * et:128 * et + 128, 0:512])
        for et in range(NET):
            nc.sync.dma_start(xts[0, et][:, 512:1024],
                              xT[0][128 * et:128 * et + 128, 512:1024])
        # second half feeds proj chunks 2-3; keep it off the rings while the
        # first chunk's weights land.
        with tc.tile_wait_until(0.02):
            for et in range(NET):
                nc.sync.dma_start(xts[0, et][:, 1024:2048],
                                  xT[0][128 * et:128 * et + 128, 1024:2048])
        # batch-1 x is not needed until ~halfway through the weave; defer
        # its transfers so they don't clog the DMA rings early (they would
        # stall the scalar queue's weight issues and delay the first exp).
        with tc.tile_wait_until(0.045):
            for et in range(NET):
                nc.sync.dma_start(xts[1, et][:],
                                  xT[1][128 * et:128 * et + 128, :])

        # gpsimd queue, continued: the rest of the constants.
        mask_sb = cpool.tile([128, 256], bf16, tag="mask")
        nc.gpsimd.dma_start(mask_sb[:], mask_d[:])
        mask_v = mask_sb[:].rearrange("p (a x) -> p a x", a=2)
        sel_sb = cpool.tile([16, 1024], bf16, tag="sel")
        nc.gpsimd.dma_start(sel_sb[:], sel_d[:])
        for et in range(NET):
            t = cpool.tile([128, DLOC], bf16, tag=f"wv{et}", name=f"w_v_{et}")
            nc.gpsimd.dma_start(t[:], wv[128 * et:128 * et + 128, :])
            w_sb["v", et] = t
        wo_sb = []
        with tc.tile_wait_until(0.08):
            for i in range(NET):
                t = cpool.tile([128, D], bf16, tag=f"wo{i}", name=f"wo{i}")
                nc.gpsimd.dma_start(t[:], woT[128 * i:128 * i + 128, :])
                wo_sb.append(t)

        # ---------------- persistent q/k/v tiles (per batch) --------------
        qt_sb = [qkvp.tile([128, S], bf16, tag=f"qt{b}", name=f"qt{b}")
                 for b in range(B)]
        kt_sb = [qkvp.tile([128, S], bf16, tag=f"kt{b}", name=f"kt{b}")
                 for b in range(B)]
        # v in [seq, head, dim] layout: [128, kt, 2 heads, 65]
        v_sb = [qkvp.tile([128, NKT * HL * VROW], bf16, tag=f"v{b}",
                          name=f"v{b}") for b in range(B)]
        v_view = [v_sb[b][:].rearrange("p (kt h x) -> p kt h x",
                                       kt=NKT, h=HL) for b in range(B)]
        for b in range(B):
            nc.vector.memset(v_view[b][:, :, :, 64:65], 1.0)

        a2a_in = dpool.tile([NCORES * HL * VROW, SSL], bf16, name="a2a_in")
        a2a_out = dpool.tile([NCORES * HL * VROW, SSL], bf16, name="a2a_out")

        with (
            tc.tile_pool(name="psc", bufs=3, space="PSUM") as pscp,
            tc.tile_pool(name="pat", bufs=1, space="PSUM") as patp,
            tc.tile_pool(name="rtmp", bufs=2) as rtp,
            tc.tile_pool(name="et", bufs=4) as etp,
            tc.tile_pool(name="stg", bufs=2) as stgp,
        ):
            # proj and scores share one 3-deep rotation of 2-bank PSUM
            # tiles (6 banks + 2 for the attention accumulators = all 8).
            def proj_qk(b, c):
                cs = slice(SC * c, SC * c + SC)
                for nm, dst in (("q", qt_sb[b]), ("k", kt_sb[b])):
                    pt = pscp.tile([128, 2 * SC], f32, tag="ps",
                                   name=f"pp{nm}_{b}_{c}")
                    p = pt[:, 0:SC]       # bank-half A: raw projection
                    p2 = pt[:, SC:2 * SC]  # bank-half B: rope partner
                    for et in range(NET):
                        nc.tensor.matmul(
                            p, w_sb[nm, et][:], xts[b, et][:, cs],
                            start=(et == 0), stop=(et == NET - 1),
                        )
                    t1c = rtp.tile([128, SC], bf16, tag="t1c",
                                   name=f"t1c{b}{c}{nm}")
                    nc.vector.tensor_copy(t1c[:], p)
                    nc.tensor.matmul(p2, perm_sb[:], t1c[:],
                                     start=True, stop=True)
                    ta = rtp.tile([128, SC], f32, tag="ra",
                                  name=f"ra{b}{c}{nm}")
                    tb = rtp.tile([128, SC], f32, tag="rb",
                                  name=f"rb{b}{c}{nm}")
                    nc.vector.tensor_mul(ta[:], cos_sb[:, cs], p)
                    nc.vector.tensor_mul(tb[:], sin_sb[:, cs], p2)
                    nc.vector.tensor_add(dst[:, cs], ta[:], tb[:])

            def proj_v(b, c):
                # v straight into [s, d] layout: stationary = x s-slices
                for j in range(SC // 128):
                    kt = (SC * c) // 128 + j
                    ss = slice(128 * kt, 128 * kt + 128)
                    pv = pscp.tile([128, 2 * SC], f32, tag="ps",
                                   name=f"ppv_{b}_{c}_{j}")
                    for et in range(NET):
                        nc.tensor.matmul(
                            pv[:, 0:DLOC], xts[b, et][:, ss],
                            w_sb["v", et][:],
                            start=(et == 0), stop=(et == NET - 1),
                        )
                    nc.vector.tensor_copy(
                        v_view[b][:, kt, :, 0:64],
                        pv[:, 0:DLOC].rearrange("p (h x) -> p h x", h=HL))

            def attn_chunk(b, qc):
                pa = {}
                for h in range(HL):
                    pa[h] = patp.tile([VROW, SC], f32, tag=f"pa{h}",
                                      name=f"pa{b}{qc}{h}")
                nkt = 4 * qc + 4
                for kt in range(nkt):
                    rel = kt - 4 * qc
                    lo = 0 if rel < 0 else 128 * rel
                    psc = pscp.tile([128, 2 * SC], f32, tag="ps",
                                    name=f"ps{b}{qc}{kt}")
                    for h in range(HL):
                        hs = slice(64 * h, 64 * h + 64)
                        nc.tensor.matmul(
                            psc[:, SC * h + lo:SC * h + SC],
                            kt_sb[b][hs, 128 * kt:128 * kt + 128],
                            qt_sb[b][hs, SC * qc + lo:SC * qc + SC],
                            start=True, stop=True,
                        )
                    et = etp.tile([128, 2 * SC], bf16, tag="et",
                                  name=f"et{b}{qc}{kt}")
                    psc_v = psc[:].rearrange("p (a x) -> p a x", a=2)
                    et_v = et[:].rearrange("p (a x) -> p a x", a=2)
                    nc.scalar.activation(et_v[:, :, lo:], psc_v[:, :, lo:],
                                         Exp)
                    if rel >= 0:
                        nc.vector.tensor_mul(
                            et_v[:, :, lo:lo + 128],
                            et_v[:, :, lo:lo + 128],
                            mask_v[:, :, 0:128])
                    for h in range(HL):
                        nc.tensor.matmul(
                            pa[h][:, lo:],
                            v_view[b][:, kt, h, :],
                            et[:, SC * h + lo:SC * h + SC],
                            start=(kt == 0), stop=(kt == nkt - 1),
                            skip_group_check=True,
                        )
                # stage this (b, qc) block straight into the a2a input:
                # dst rank r = 4*b + qc gets rows [130r, 130r+130), laid out
                # as 128 contiguous dim-rows (h0 then h1) + 2 denominator
                # rows, so the consumer's g loads are plain 2D DMAs.
                stg = stgp.tile([VROW, 2 * SC], bf16, tag="stg",
                                name=f"stg{b}{qc}")
                for h in range(HL):
                    nc.vector.tensor_copy(stg[:, SC * h:SC * h + SC],
                                          pa[h][:])
                r = 4 * b + qc
                base = HL * VROW * r
                nc.sync.dma_start(
                    a2a_in[base:base + 128, :].rearrange(
                        "(a p) x -> p a x", a=2),
                    stg[0:64, :].rearrange("p (a x) -> p a x", a=2))
                nc.sync.dma_start(
                    a2a_in[base + 128:base + 130, :].rearrange(
                        "(p a) x -> p a x", p=1),
                    stg[64:65, :].rearrange("p (a x) -> p a x", a=2))

            # ---------------- interleaved schedule ----------------
            # attention instructions get a priority boost so the scores ->
            # exp stream stays dense; projection matmuls fill the PE gaps.
            def attn_hi(b, qc):
                # offset larger than the whole program's instruction count:
                # every attention instruction outranks every projection
                # instruction in the scheduler's ready heap, while the
                # attention chunks keep their relative order.
                with tc.high_priority(offset=1000):
                    attn_chunk(b, qc)

            def proj_chunk(b, c):
                proj_qk(b, c)
                proj_v(b, c)

            proj_chunk(0, 0)
            proj_chunk(0, 1)
            attn_hi(0, 0)
            proj_chunk(0, 2)
            attn_hi(0, 1)
            proj_chunk(0, 3)
            attn_hi(0, 2)
            proj_chunk(1, 0)
            attn_hi(0, 3)
            proj_chunk(1, 1)
            attn_hi(1, 0)
            proj_chunk(1, 2)
            attn_hi(1, 1)
            proj_chunk(1, 3)
            attn_hi(1, 2)
            attn_hi(1, 3)

        nc.gpsimd.collective_compute(
            "AllToAll",
            mybir.AluOpType.bypass,
            replica_groups=[list(range(NCORES))],
            ins=[a2a_in.opt()],
            outs=[a2a_out.opt()],
        )

        # ---------------- normalize + output projection ----------------
        with (
            tc.tile_pool(name="late", bufs=1) as lpool,
            tc.tile_pool(name="ocp", bufs=2) as ocp,
            tc.tile_pool(name="po", bufs=1, space="PSUM") as pop,
            tc.tile_pool(name="pb", bufs=4, space="PSUM") as pbp,
        ):
            # a2a_out rows per src rank r: 128 dim-rows (h-major) + 2 denom
            # rows, cols SSL.
            a2a_v = a2a_out[:].rearrange("(r y) q -> r y q", y=HL * VROW)
            denom = lpool.tile([16, SSL], bf16, tag="den", name="denom")
            nc.scalar.dma_start(denom[:], a2a_v[:, 128:130, :])
            g_tiles = []
            g_engines = [nc.scalar, nc.sync, nc.gpsimd]
            for i in range(NET):
                g = lpool.tile([128, SSL], bf16, tag=f"g{i}", name=f"g{i}")
                # e-tile i = global heads 2i, 2i+1 = src rank i, h 0..1
                g_engines[i % 3].dma_start(
                    g[:], a2a_out[HL * VROW * i:HL * VROW * i + 128, :])
                g_tiles.append(g)

            denf = lpool.tile([16, SSL], f32, tag="denf", name="denf")
            nc.vector.tensor_copy(denf[:], denom[:])
            recf = lpool.tile([16, SSL], f32, tag="recf", name="recf")
            nc.vector.reciprocal_approx_fast(recf[:], denf[:])
            recip = lpool.tile([16, SSL], bf16, tag="rec", name="recip")
            nc.vector.tensor_copy(recip[:], recf[:])

            pbs = []
            for i in range(NET):
                pb = pbp.tile([128, SSL], f32, tag="pb", name=f"pb{i}")
                nc.tensor.matmul(pb[:], sel_sb[:, 128 * i:128 * i + 128],
                                 recip[:], start=True, stop=True)
                pbs.append(pb)
            norm = []
            for i in range(NET):
                nv = lpool.tile([128, SSL], bf16, tag=f"n{i}", name=f"n{i}")
                nc.vector.tensor_mul(nv[:], g_tiles[i][:], pbs[i][:])
                norm.append(nv)

            # i-outer accumulation: the first matmuls need only norm[0],
            # so the projection overlaps norm production.
            for ec in range(2):
                pos = [pop.tile([128, SC], f32, tag=f"po{st}",
                                name=f"po{ec}{st}")
                       for st in range(SSL // 128)]
                for i in range(NET):
                    for st in range(SSL // 128):
                        nc.tensor.matmul(
                            pos[st][:],
                            norm[i][:, 128 * st:128 * st + 128],
                            wo_sb[i][:, SC * ec:SC * ec + SC],
                            start=(i == 0), stop=(i == NET - 1),
                        )
                for st in range(SSL // 128):
                    oc = ocp.tile([128, SC], f32, tag="oc",
                                  name=f"oc{ec}{st}")
                    nc.vector.tensor_copy(oc[:], pos[st][:])
                    eng = nc.sync if st % 2 == 0 else nc.scalar
                    eng.dma_start(
                        out[128 * st:128 * st + 128, SC * ec:SC * ec + SC],
                        oc[:])


def _host_prep(x, Wq, Wk, Wv, Wo):
    import ml_dtypes
    bf = ml_dtypes.bfloat16

    x = np.asarray(x, np.float32)
    Wq = np.asarray(Wq, np.float32)
    Wk = np.asarray(Wk, np.float32)
    Wv = np.asarray(Wv, np.float32)
    Wo = np.asarray(Wo, np.float32)

    perm = np.concatenate([np.arange(0, DK, 2), np.arange(1, DK, 2)])
    swap = np.concatenate([np.arange(32, 64), np.arange(0, 32)])
    swap128 = np.concatenate([swap, 64 + swap])

    freqs = 1.0 / (ROPE_THETA ** (np.arange(0, DK, 2, dtype=np.float64) / DK))
    ang = np.arange(S, dtype=np.float64)[:, None] * freqs[None, :]  # [S, 32]
    cos32 = np.cos(ang).T.astype(np.float32)  # [32, S]
    sin32 = np.sin(ang).T.astype(np.float32)
    cos_t = np.ascontiguousarray(np.tile(cos32, (4, 1))).astype(bf)
    sin_t = np.ascontiguousarray(
        np.concatenate([-sin32, sin32, -sin32, sin32], axis=0)).astype(bf)

    tri = np.triu(np.ones((128, 128), np.float32))
    mask256 = np.ascontiguousarray(
        np.concatenate([tri, tri], axis=1)).astype(bf)

    sel = np.zeros((16, 1024), np.float32)
    for i in range(NET):
        for m in range(128):
            sel[2 * i + m // 64, 128 * i + m] = 1.0

    permM = np.zeros((128, 128), np.float32)
    for r_ in range(128):
        permM[swap128[r_], r_] = 1.0

    xTb = [np.ascontiguousarray(x[b].T).astype(bf) for b in range(B)]
    woT = np.ascontiguousarray(Wo.T).astype(bf)

    scale = 1.0 / np.sqrt(np.float32(DK))
    in_maps = []
    for c in range(NCORES):
        rows = np.concatenate(
            [DK * (2 * c) + perm, DK * (2 * c + 1) + perm])
        wq_c = np.ascontiguousarray((scale * Wq[rows, :]).T)
        wk_c = np.ascontiguousarray(Wk[rows, :].T)
        wv_c = np.ascontiguousarray(
            Wv[DLOC * c:DLOC * c + DLOC, :].T).astype(bf)
        in_maps.append({
            "xT0": xTb[0], "xT1": xTb[1],
            "wq": wq_c.astype(bf),
            "wk": wk_c.astype(bf),
            "wv": wv_c,
            "woT": woT,
            "cos_t": cos_t, "sin_t": sin_t,
            "perm128": permM.astype(bf),
            "mask256": mask256, "sel16": sel.astype(bf),
        })
    return in_maps


def _assemble(results):
    full = np.empty((B, S, D), np.float32)
    for r_ in range(NCORES):
        full[r_ // 4, SSL * (r_ % 4):SSL * (r_ % 4) + SSL, :] = \
            results[r_]["out"]
    return full


def kernel(x, Wq, Wk, Wv, Wo):
    if "nc" not in _CACHE:
        _CACHE["nc"] = _build_nc()
    nc = _CACHE["nc"]
    in_maps = _host_prep(x, Wq, Wk, Wv, Wo)

    if os.environ.get("MHA_SIM"):
        from concourse.bass_interp import MultiCoreSim
        sim = MultiCoreSim(nc, num_cores=NCORES)
        for c in range(NCORES):
            for k, v in in_maps[c].items():
                sim.cores[c].tensor(k)[:] = v
        sim.simulate()
        results = [{"out": np.array(sim.cores[c].mem_tensor("out"))}
                   for c in range(NCORES)]
        return _assemble(results)

    trace = bool(os.environ.get("MHA_TRACE"))
    res = run_bass_kernel_spmd(
        nc, in_maps, list(range(NCORES)), trace=trace)
    LAST_EXEC_NS["ns"] = res.exec_time_ns
    return _assemble(res.results)


# revision 35
# speedup vs baseline: 1.1167x; 1.1167x over previous
"""Causal MHSA (B=2, S=2048, D=1024, H=16, RoPE) on 8 TRN2 NeuronCores.

Sharding: each core owns 2 heads x both batches (tensor parallel on
heads). Attention runs in transposed ("scores^T") layout so no on-device
transposes of Q/K/attn are needed; softmax denominators ride as an extra
ones-column in V. An 8-rank AllToAll re-shards from head-parallel to
sequence-parallel, after which each core runs the output projection for
its (batch, seq-slice) and emits a [512, 1024] output slice.

All matmul operands are bf16 (fp32 accumulation in PSUM); q/k stay bf16
through RoPE so the per-head score matmuls run row-grouped and
concurrently at full bf16 rate. V is projected directly into [seq, dim]
layout (x s-chunks as the stationary operand), removing PE transposes.
Projection chunks and attention q-chunks are interleaved in program
order so the PE has dense work while the scalar engine (exp) grinds
through softmax; attention outputs are staged into the AllToAll input
eagerly per (batch, q-chunk) so the collective triggers immediately
after the last chunk. Input DMAs are split across the Sync/Scalar/GpSimd
queues and ordered so the first projection chunk starts early.
"""

import os
import numpy as np

import concourse.bass as bass
import concourse.bacc as bacc
import concourse.mybir as mybir
import concourse.tile as tile
from concourse.bass_utils import run_bass_kernel_spmd

B, S, D, H, DK = 2, 2048, 1024, 16, 64
NCORES = 8
HL = 2            # heads per core
DLOC = HL * DK    # 128 local head dims
SC = 512          # q-chunk / moving free dim
NQC = S // SC     # 4 q-chunks
NKT = S // 128    # 16 k-tiles
NET = D // 128    # 8 e-tiles (contraction over embedding)
SSL = S // 4      # 512-row seq slice per rank (within a batch)
VROW = 65         # 64 dims + denominator ones-row
ROPE_THETA = 10000.0

f32 = mybir.dt.float32
bf16 = mybir.dt.bfloat16

LAST_EXEC_NS = {"ns": None}
_CACHE = {}


def _build_nc():
    nc = bacc.Bacc(
        "TRN2", target_bir_lowering=False, debug=False, num_devices=NCORES
    )

    xT = [
        nc.dram_tensor(f"xT{b}", [D, S], bf16, kind="ExternalInput").ap()
        for b in range(B)
    ]
    wq = nc.dram_tensor("wq", [D, DLOC], bf16, kind="ExternalInput").ap()
    wk = nc.dram_tensor("wk", [D, DLOC], bf16, kind="ExternalInput").ap()
    wv = nc.dram_tensor("wv", [D, DLOC], bf16, kind="ExternalInput").ap()
    woT = nc.dram_tensor("woT", [D, D], bf16, kind="ExternalInput").ap()
    cos_d = nc.dram_tensor("cos_t", [128, S], bf16, kind="ExternalInput").ap()
    sin_d = nc.dram_tensor("sin_t", [128, S], bf16, kind="ExternalInput").ap()
    perm_d = nc.dram_tensor("perm128", [128, 128], bf16,
                            kind="ExternalInput").ap()
    mask_d = nc.dram_tensor("mask256", [128, 256], bf16,
                            kind="ExternalInput").ap()
    sel_d = nc.dram_tensor("sel16", [16, 1024], bf16, kind="ExternalInput").ap()
    out = nc.dram_tensor("out", [SSL, D], f32, kind="ExternalOutput").ap()

    with tile.TileContext(nc) as tc:
        _body(nc, tc, xT, wq, wk, wv, woT, cos_d, sin_d, perm_d, mask_d,
              sel_d, out)

    nc.compile()
    return nc


def _body(nc, tc, xT, wq, wk, wv, woT, cos_d, sin_d, perm_d, mask_d,
          sel_d, out):
    Exp = mybir.ActivationFunctionType.Exp

    with (
        tc.tile_pool(name="const", bufs=1) as cpool,
        tc.tile_pool(name="xw", bufs=1) as xwpool,
        tc.tile_pool(name="qkv", bufs=1) as qkvp,
        tc.tile_pool(name="dram", bufs=1, space="DRAM") as dpool,
    ):
        # ---------------- input DMAs, spread across queues ----------------
        # gpsimd queue: q/k weights first (needed by proj chunk 0), then
        # rope tables. The scalar queue is kept free of early DMA issues so
        # the exp stream starts as soon as the first scores land.
        w_sb = {}
        cos_sb = cpool.tile([128, S], bf16, tag="cos")
        sin_sb = cpool.tile([128, S], bf16, tag="sin")
        for et in range(NET):
            t = cpool.tile([128, DLOC], bf16, tag=f"wq{et}", name=f"w_q_{et}")
            nc.gpsimd.dma_start(t[:], wq[128 * et:128 * et + 128, :])
            w_sb["q", et] = t
        # rope tables arrive chunk 0 first: only 2x128KB sits on the
        # critical path to the first scores instead of the full 1MB.
        nc.gpsimd.dma_start(cos_sb[:, 0:SC], cos_d[:, 0:SC])
        nc.gpsimd.dma_start(sin_sb[:, 0:SC], sin_d[:, 0:SC])
        perm_sb = cpool.tile([128, 128], bf16, tag="perm")
        nc.gpsimd.dma_start(perm_sb[:], perm_d[:])
        for et in range(NET):
            t = cpool.tile([128, DLOC], bf16, tag=f"wk{et}", name=f"w_k_{et}")
            nc.gpsimd.dma_start(t[:], wk[128 * et:128 * et + 128, :])
            w_sb["k", et] = t
        for cc_ in range(1, NQC):
            csl = slice(SC * cc_, SC * cc_ + SC)
            nc.gpsimd.dma_start(cos_sb[:, csl], cos_d[:, csl])
            nc.gpsimd.dma_start(sin_sb[:, csl], sin_d[:, csl])

        # sync queue: batch-0 x in half-row chunks (proj chunk 0 early).
        xts = {}
        for b in range(B):
            for et in range(NET):
                t = xwpool.tile([128, S], bf16, tag=f"xt{b}{et}",
                                name=f"xt{b}_{et}")
                xts[b, et] = t
        # chunk 0 alone first: it is the only x data on the critical path to
        # the first scores/exp, and the rings are bandwidth-bound early.
        for et in range(NET):
            nc.sync.dma_start(xts[0, et][:, 0:512],
                              xT[0][128 * et:128 * et + 128, 0:512])
        for et in range(NET):
            nc.sync.dma_start(xts[0, et][:, 512:1024],
                              xT[0][128 * et:128 * et + 128, 512:1024])
        # second half feeds proj chunks 2-3; keep it off the rings while the
        # first chunk's weights land.
        with tc.tile_wait_until(0.02):
            for et in range(NET):
                nc.sync.dma_start(xts[0, et][:, 1024:2048],
                                  xT[0][128 * et:128 * et + 128, 1024:2048])
        # batch-1 x is not needed until ~halfway through the weave; defer
        # its transfers so they don't clog the DMA rings early (they would
        # stall the scalar queue's weight issues and delay the first exp).
        with tc.tile_wait_until(0.045):
            for et in range(NET):
                nc.sync.dma_start(xts[1, et][:],
                                  xT[1][128 * et:128 * et + 128, :])

        # gpsimd queue, continued: the rest of the constants.
        mask_sb = cpool.tile([128, 256], bf16, tag="mask")
        nc.gpsimd.dma_start(mask_sb[:], mask_d[:])
        mask_v = mask_sb[:].rearrange("p (a x) -> p a x", a=2)
        sel_sb = cpool.tile([16, 1024], bf16, tag="sel")
        nc.gpsimd.dma_start(sel_sb[:], sel_d[:])
        for et in range(NET):
            t = cpool.tile([128, DLOC], bf16, tag=f"wv{et}", name=f"w_v_{et}")
            nc.gpsimd.dma_start(t[:], wv[128 * et:128 * et + 128, :])
            w_sb["v", et] = t
        wo_sb = []
        with tc.tile_wait_until(0.08):
            for i in range(NET):
                t = cpool.tile([128, D], bf16, tag=f"wo{i}", name=f"wo{i}")
                nc.gpsimd.dma_start(t[:], woT[128 * i:128 * i + 128, :])
                wo_sb.append(t)

        # ---------------- persistent q/k/v tiles (per batch) --------------
        qt_sb = [qkvp.tile([128, S], bf16, tag=f"qt{b}", name=f"qt{b}")
                 for b in range(B)]
        kt_sb = [qkvp.tile([128, S], bf16, tag=f"kt{b}", name=f"kt{b}")
                 for b in range(B)]
        # v in [seq, head, dim] layout: [128, kt, 2 heads, 65]
        v_sb = [qkvp.tile([128, NKT * HL * VROW], bf16, tag=f"v{b}",
                          name=f"v{b}") for b in range(B)]
        v_view = [v_sb[b][:].rearrange("p (kt h x) -> p kt h x",
                                       kt=NKT, h=HL) for b in range(B)]
        for b in range(B):
            nc.vector.memset(v_view[b][:, :, :, 64:65], 1.0)

        a2a_in = dpool.tile([NCORES * HL * VROW, SSL], bf16, name="a2a_in")
        a2a_out = dpool.tile([NCORES * HL * VROW, SSL], bf16, name="a2a_out")

        with (
            tc.tile_pool(name="psc", bufs=3, space="PSUM") as pscp,
            tc.tile_pool(name="pat", bufs=1, space="PSUM") as patp,
            tc.tile_pool(name="rtmp", bufs=2) as rtp,
            tc.tile_pool(name="et", bufs=6) as etp,
            tc.tile_pool(name="stg", bufs=2) as stgp,
        ):
            # proj and scores share one 3-deep rotation of 2-bank PSUM
            # tiles (6 banks + 2 for the attention accumulators = all 8).
            def proj_qk(b, c):
                cs = slice(SC * c, SC * c + SC)
                for nm, dst in (("q", qt_sb[b]), ("k", kt_sb[b])):
                    pt = pscp.tile([128, 2 * SC], f32, tag="ps",
                                   name=f"pp{nm}_{b}_{c}")
                    p = pt[:, 0:SC]       # bank-half A: raw projection
                    p2 = pt[:, SC:2 * SC]  # bank-half B: rope partner
                    for et in range(NET):
                        nc.tensor.matmul(
                            p, w_sb[nm, et][:], xts[b, et][:, cs],
                            start=(et == 0), stop=(et == NET - 1),
                        )
                    t1c = rtp.tile([128, SC], bf16, tag="t1c",
                                   name=f"t1c{b}{c}{nm}")
                    nc.vector.tensor_copy(t1c[:], p)
                    nc.tensor.matmul(p2, perm_sb[:], t1c[:],
                                     start=True, stop=True)
                    ta = rtp.tile([128, SC], f32, tag="ra",
                                  name=f"ra{b}{c}{nm}")
                    tb = rtp.tile([128, SC], f32, tag="rb",
                                  name=f"rb{b}{c}{nm}")
                    nc.vector.tensor_mul(ta[:], cos_sb[:, cs], p)
                    nc.vector.tensor_mul(tb[:], sin_sb[:, cs], p2)
                    nc.vector.tensor_add(dst[:, cs], ta[:], tb[:])

            def proj_v(b, c):
                # v straight into [s, d] layout: stationary = x s-slices
                for j in range(SC // 128):
                    kt = (SC * c) // 128 + j
                    ss = slice(128 * kt, 128 * kt + 128)
                    pv = pscp.tile([128, 2 * SC], f32, tag="ps",
                                   name=f"ppv_{b}_{c}_{j}")
                    for et in range(NET):
                        nc.tensor.matmul(
                            pv[:, 0:DLOC], xts[b, et][:, ss],
                            w_sb["v", et][:],
                            start=(et == 0), stop=(et == NET - 1),
                        )
                    nc.vector.tensor_copy(
                        v_view[b][:, kt, :, 0:64],
                        pv[:, 0:DLOC].rearrange("p (h x) -> p h x", h=HL))

            def attn_chunk(b, qc):
                pa = {}
                for h in range(HL):
                    pa[h] = patp.tile([VROW, SC], f32, tag=f"pa{h}",
                                      name=f"pa{b}{qc}{h}")
                nkt = 4 * qc + 4
                for kt in range(nkt):
                    rel = kt - 4 * qc
                    lo = 0 if rel < 0 else 128 * rel
                    psc = pscp.tile([128, 2 * SC], f32, tag="ps",
                                    name=f"ps{b}{qc}{kt}")
                    for h in range(HL):
                        hs = slice(64 * h, 64 * h + 64)
                        nc.tensor.matmul(
                            psc[:, SC * h + lo:SC * h + SC],
                            kt_sb[b][hs, 128 * kt:128 * kt + 128],
                            qt_sb[b][hs, SC * qc + lo:SC * qc + SC],
                            start=True, stop=True,
                        )
                    et = etp.tile([128, 2 * SC], bf16, tag="et",
                                  name=f"et{b}{qc}{kt}")
                    psc_v = psc[:].rearrange("p (a x) -> p a x", a=2)
                    et_v = et[:].rearrange("p (a x) -> p a x", a=2)
                    nc.scalar.activation(et_v[:, :, lo:], psc_v[:, :, lo:],
                                         Exp)
                    if rel >= 0:
                        nc.vector.tensor_mul(
                            et_v[:, :, lo:lo + 128],
                            et_v[:, :, lo:lo + 128],
                            mask_v[:, :, 0:128])
                    for h in range(HL):
                        nc.tensor.matmul(
                            pa[h][:, lo:],
                            v_view[b][:, kt, h, :],
                            et[:, SC * h + lo:SC * h + SC],
                            start=(kt == 0), stop=(kt == nkt - 1),
                            skip_group_check=True,
                        )
                # stage this (b, qc) block straight into the a2a input:
                # dst rank r = 4*b + qc gets rows [130r, 130r+130), laid out
                # as 128 contiguous dim-rows (h0 then h1) + 2 denominator
                # rows, so the consumer's g loads are plain 2D DMAs.
                stg = stgp.tile([VROW, 2 * SC], bf16, tag="stg",
                                name=f"stg{b}{qc}")
                for h in range(HL):
                    nc.vector.tensor_copy(stg[:, SC * h:SC * h + SC],
                                          pa[h][:])
                r = 4 * b + qc
                base = HL * VROW * r
                nc.sync.dma_start(
                    a2a_in[base:base + 128, :].rearrange(
                        "(a p) x -> p a x", a=2),
                    stg[0:64, :].rearrange("p (a x) -> p a x", a=2))
                nc.sync.dma_start(
                    a2a_in[base + 128:base + 130, :].rearrange(
                        "(p a) x -> p a x", p=1),
                    stg[64:65, :].rearrange("p (a x) -> p a x", a=2))

            # ---------------- interleaved schedule ----------------
            # attention instructions get a priority boost so the scores ->
            # exp stream stays dense; projection matmuls fill the PE gaps.
            def attn_hi(b, qc):
                # offset larger than the whole program's instruction count:
                # every attention instruction outranks every projection
                # instruction in the scheduler's ready heap, while the
                # attention chunks keep their relative order.
                with tc.high_priority(offset=1000):
                    attn_chunk(b, qc)

            def proj_chunk(b, c):
                proj_qk(b, c)
                proj_v(b, c)

            proj_chunk(0, 0)
            proj_chunk(0, 1)
            attn_hi(0, 0)
            proj_chunk(0, 2)
            attn_hi(0, 1)
            proj_chunk(0, 3)
            attn_hi(0, 2)
            proj_chunk(1, 0)
            attn_hi(0, 3)
            proj_chunk(1, 1)
            attn_hi(1, 0)
            proj_chunk(1, 2)
            attn_hi(1, 1)
            proj_chunk(1, 3)
            attn_hi(1, 2)
            attn_hi(1, 3)

        nc.gpsimd.collective_compute(
            "AllToAll",
            mybir.AluOpType.bypass,
            replica_groups=[list(range(NCORES))],
            ins=[a2a_in.opt()],
            outs=[a2a_out.opt()],
        )

        # ---------------- normalize + output projection ----------------
        with (
            tc.tile_pool(name="late", bufs=1) as lpool,
            tc.tile_pool(name="ocp", bufs=2) as ocp,
            tc.tile_pool(name="po", bufs=1, space="PSUM") as pop,
            tc.tile_pool(name="pb", bufs=4, space="PSUM") as pbp,
        ):
            # a2a_out rows per src rank r: 128 dim-rows (h-major) + 2 denom
            # rows, cols SSL.
            a2a_v = a2a_out[:].rearrange("(r y) q -> r y q", y=HL * VROW)
            denom = lpool.tile([16, SSL], bf16, tag="den", name="denom")
            nc.scalar.dma_start(denom[:], a2a_v[:, 128:130, :])
            g_tiles = []
            g_engines = [nc.scalar, nc.sync, nc.gpsimd]
            for i in range(NET):
                g = lpool.tile([128, SSL], bf16, tag=f"g{i}", name=f"g{i}")
                # e-tile i = global heads 2i, 2i+1 = src rank i, h 0..1
                g_engines[i % 3].dma_start(
                    g[:], a2a_out[HL * VROW * i:HL * VROW * i + 128, :])
                g_tiles.append(g)

            denf = lpool.tile([16, SSL], f32, tag="denf", name="denf")
            nc.vector.tensor_copy(denf[:], denom[:])
            recf = lpool.tile([16, SSL], f32, tag="recf", name="recf")
            nc.vector.reciprocal_approx_fast(recf[:], denf[:])
            recip = lpool.tile([16, SSL], bf16, tag="rec", name="recip")
            nc.vector.tensor_copy(recip[:], recf[:])

            pbs = []
            for i in range(NET):
                pb = pbp.tile([128, SSL], f32, tag="pb", name=f"pb{i}")
                nc.tensor.matmul(pb[:], sel_sb[:, 128 * i:128 * i + 128],
                                 recip[:], start=True, stop=True)
                pbs.append(pb)
            norm = []
            for i in range(NET):
                nv = lpool.tile([128, SSL], bf16, tag=f"n{i}", name=f"n{i}")
                nc.vector.tensor_mul(nv[:], g_tiles[i][:], pbs[i][:])
                norm.append(nv)

            # i-outer accumulation: the first matmuls need only norm[0],
            # so the projection overlaps norm production.
            for ec in range(2):
                pos = [pop.tile([128, SC], f32, tag=f"po{st}",
                                name=f"po{ec}{st}")
                       for st in range(SSL // 128)]
                for i in range(NET):
                    for st in range(SSL // 128):
                        nc.tensor.matmul(
                            pos[st][:],
                            norm[i][:, 128 * st:128 * st + 128],
                            wo_sb[i][:, SC * ec:SC * ec + SC],
                            start=(i == 0), stop=(i == NET - 1),
                        )
                for st in range(SSL // 128):
                    oc = ocp.tile([128, SC], f32, tag="oc",
                                  name=f"oc{ec}{st}")
                    nc.vector.tensor_copy(oc[:], pos[st][:])
                    eng = nc.sync if st % 2 == 0 else nc.scalar
                    eng.dma_start(
                        out[128 * st:128 * st + 128, SC * ec:SC * ec + SC],
                        oc[:])


def _host_prep(x, Wq, Wk, Wv, Wo):
    import ml_dtypes
    bf = ml_dtypes.bfloat16

    x = np.asarray(x, np.float32)
    Wq = np.asarray(Wq, np.float32)
    Wk = np.asarray(Wk, np.float32)
    Wv = np.asarray(Wv, np.float32)
    Wo = np.asarray(Wo, np.float32)

    perm = np.concatenate([np.arange(0, DK, 2), np.arange(1, DK, 2)])
    swap = np.concatenate([np.arange(32, 64), np.arange(0, 32)])
    swap128 = np.concatenate([swap, 64 + swap])

    freqs = 1.0 / (ROPE_THETA ** (np.arange(0, DK, 2, dtype=np.float64) / DK))
    ang = np.arange(S, dtype=np.float64)[:, None] * freqs[None, :]  # [S, 32]
    cos32 = np.cos(ang).T.astype(np.float32)  # [32, S]
    sin32 = np.sin(ang).T.astype(np.float32)
    cos_t = np.ascontiguousarray(np.tile(cos32, (4, 1))).astype(bf)
    sin_t = np.ascontiguousarray(
        np.concatenate([-sin32, sin32, -sin32, sin32], axis=0)).astype(bf)

    tri = np.triu(np.ones((128, 128), np.float32))
    mask256 = np.ascontiguousarray(
        np.concatenate([tri, tri], axis=1)).astype(bf)

    sel = np.zeros((16, 1024), np.float32)
    for i in range(NET):
        for m in range(128):
            sel[2 * i + m // 64, 128 * i + m] = 1.0

    permM = np.zeros((128, 128), np.float32)
    for r_ in range(128):
        permM[swap128[r_], r_] = 1.0

    xTb = [np.ascontiguousarray(x[b].T).astype(bf) for b in range(B)]
    woT = np.ascontiguousarray(Wo.T).astype(bf)

    scale = 1.0 / np.sqrt(np.float32(DK))
    in_maps = []
    for c in range(NCORES):
        rows = np.concatenate(
            [DK * (2 * c) + perm, DK * (2 * c + 1) + perm])
        wq_c = np.ascontiguousarray((scale * Wq[rows, :]).T)
        wk_c = np.ascontiguousarray(Wk[rows, :].T)
        wv_c = np.ascontiguousarray(
            Wv[DLOC * c:DLOC * c + DLOC, :].T).astype(bf)
        in_maps.append({
            "xT0": xTb[0], "xT1": xTb[1],
            "wq": wq_c.astype(bf),
            "wk": wk_c.astype(bf),
            "wv": wv_c,
            "woT": woT,
            "cos_t": cos_t, "sin_t": sin_t,
            "perm128": permM.astype(bf),
            "mask256": mask256, "sel16": sel.astype(bf),
        })
    return in_maps


def _assemble(results):
    full = np.empty((B, S, D), np.float32)
    for r_ in range(NCORES):
        full[r_ // 4, SSL * (r_ % 4):SSL * (r_ % 4) + SSL, :] = \
            np.asarray(results[r_]["out"], dtype=np.float32)
    return full


def kernel(x, Wq, Wk, Wv, Wo):
    if "nc" not in _CACHE:
        _CACHE["nc"] = _build_nc()
    nc = _CACHE["nc"]
    in_maps = _host_prep(x, Wq, Wk, Wv, Wo)

    if os.environ.get("MHA_SIM"):
        from concourse.bass_interp import MultiCoreSim
        sim = MultiCoreSim(nc, num_cores=NCORES)
        for c in range(NCORES):
            for k, v in in_maps[c].items():
                sim.cores[c].tensor(k)[:] = v
        sim.simulate()
        results = [{"out": np.array(sim.cores[c].mem_tensor("out"))}
                   for c in range(NCORES)]
        return _assemble(results)

    trace = bool(os.environ.get("MHA_TRACE"))
    res = run_bass_kernel_spmd(
        nc, in_maps, list(range(NCORES)), trace=trace)
    LAST_EXEC_NS["ns"] = res.exec_time_ns
    return _assemble(res.results)
